# revision 4
# baseline (speedup 1.0000x reference)
"""Trainium2 Bass kernel for nn_Decoder: out = (x - b_pre) @ W^T.

Shapes (hardcoded): x [8192, 32768] f32, W [768, 32768] f32, b_pre [32768]
-> out [8192, 768] f32.

Sharding: data-parallel over the 8192 token rows across 8 NeuronCores
(1024 rows each), W replicated. The TensorE contracts over the partition
axis, so operands are host-transposed so the contraction dim d is on
partitions.

Precision/speed strategy (gate: rel err < 2e-2 on max|err|/max|ref|):
the contraction is split between two input dtypes.
  - C8 d-chunks of 256 rows run as fp8 e4m3 matmuls in DoubleRow perf
    mode: both operands pack 2 contraction rows per partition along the
    free axis, and the PE retires 2 output cols/cycle - 2x the MAC rate
    of fp16 (32768 MAC/cyc vs 16384). e4m3 quantization error for this
    data is ~3.7e-2 end-to-end if the WHOLE contraction ran fp8, and
    scales with sqrt(f) for a fraction f, so C8 is chosen to keep total
    error ~1.6e-2. W (std 0.0055) sits in e4m3's subnormal range, so W
    is pre-scaled by 128 before quantization and the fp8 partial sums
    are multiplied by 1/128 in the (fused) PSUM->SBUF accumulate.
  - The remaining d-chunks (128 rows) run as fp16 matmuls (~3e-4
    error contribution, 1 col/cycle - still faster than the f32r
    baseline's measured ~9/8 cyc/col, and fp16 LDWEIGHTS is 4x faster
    than f32r's, which the f32r baseline was co-bottlenecked on).
Both paths accumulate into the same [128, 768] f32 SBUF C tiles via
DVE adds (fp8 supers use a fused scalar_tensor_tensor multiply-add for
the 1/128 descale). All tensors touch HBM exactly once, pre-packed on
the host so every DMA is a full-partition contiguous transfer.

fp8 packing: a 256-row d-chunk maps to SBUF [128 parts, 2, n] with
contraction row d = 2p + i -> on the host this is a pure reshape of the
[d, n]-transposed tensor. PSUM accumulation-group rule: start=True
lazily zeroes the full 2KB PSUM bank, so per bank exactly one chain
starts (first 256-col region) and one stops (last region); DoubleRow
outputs are 64 partitions, so fp8 psum tiles are [64, 768] per token
half (verified bit-exact vs the quantized host reference in CoreSim).
"""

import os
import sys

if "/opt/trn_rl_repo" not in sys.path:
    sys.path.insert(0, "/opt/trn_rl_repo")

import numpy as np

N_TOK = 8192
D_IN = 32768
D_OUT = 768
N_CORES = 8
N_SHARD = N_TOK // N_CORES          # 1024 token rows per core
P = 128

# Number of 256-row fp8 d-chunks (the rest of the 32768 contraction runs
# fp16). 24 chunks = 6144 rows = 18.75% of K -> measured end-to-end rel
# err ~1.6e-2 against the 2e-2 gate. 0 disables the fp8 path entirely.
C8 = int(os.environ.get("KERNEL_FP8_CHUNKS", "24"))
D8 = C8 * 256
D16 = D_IN - D8
C16 = D16 // P                      # fp16 d-chunks of 128 rows

X8_BUFS = int(os.environ.get("KERNEL_X8_BUFS", "10"))
W8_BUFS = int(os.environ.get("KERNEL_W8_BUFS", "10"))
X16_BUFS = int(os.environ.get("KERNEL_X16_BUFS", "24"))
W16_BUFS = int(os.environ.get("KERNEL_W16_BUFS", "24"))
PS8_BUFS = int(os.environ.get("KERNEL_PS8_BUFS", "2"))
PS16_BUFS = int(os.environ.get("KERNEL_PS16_BUFS", "2"))
SUP8 = 8                            # fp8 chunks per super
SUP16 = 16                          # fp16 chunks per super

LAST_RESULTS = None  # BassKernelResults of the most recent kernel() call


def _build_bass(c8):
    import concourse.mybir as mybir
    import concourse.tile as tile
    from concourse import bacc

    fp8 = mybir.dt.float8e4
    fp16 = mybir.dt.float16
    f32 = mybir.dt.float32
    c16 = (D_IN - c8 * 256) // P
    NCH = N_SHARD // P              # 8 output row-chunks of 128 tokens

    nc = bacc.Bacc(None, target_bir_lowering=False)
    if c8:
        xP8 = nc.dram_tensor("xP8", [c8, P, 2, N_SHARD], fp8,
                             kind="ExternalInput")
        wP8 = nc.dram_tensor("wP8", [c8, P, 2, D_OUT], fp8,
                             kind="ExternalInput")
    if c16:
        xT16 = nc.dram_tensor("xT16", [c16 * P, N_SHARD], fp16,
                              kind="ExternalInput")
        wT16 = nc.dram_tensor("wT16", [c16 * P, D_OUT], fp16,
                              kind="ExternalInput")
    out = nc.dram_tensor("out", [N_SHARD, D_OUT], f32,
                         kind="ExternalOutput")

    from contextlib import ExitStack

    with tile.TileContext(nc) as tc, ExitStack() as es:
        if c8:
            x8pool = es.enter_context(tc.tile_pool(name="x8", bufs=X8_BUFS))
            w8pool = es.enter_context(tc.tile_pool(name="w8", bufs=W8_BUFS))
            ppool8 = es.enter_context(
                tc.tile_pool(name="psum8", bufs=PS8_BUFS, space="PSUM"))
        if c16:
            x16pool = es.enter_context(
                tc.tile_pool(name="x16", bufs=X16_BUFS))
            w16pool = es.enter_context(
                tc.tile_pool(name="w16", bufs=W16_BUFS))
            ppool16 = es.enter_context(
                tc.tile_pool(name="psum16", bufs=PS16_BUFS, space="PSUM"))
        cpool = es.enter_context(tc.tile_pool(name="c", bufs=1))
        if True:
            cts = [
                cpool.tile([P, D_OUT], f32, name=f"c{i}") for i in range(NCH)
            ]
            first = True

            # fp8 DoubleRow supers (chunks of 256 d-rows)
            for s0 in range(0, c8, SUP8):
                js = list(range(s0, min(s0 + SUP8, c8)))
                xts, wts = [], []
                for j in js:
                    xt = x8pool.tile([P, 2, N_SHARD], fp8, name="xt8")
                    wt = w8pool.tile([P, 2, D_OUT], fp8, name="wt8")
                    nc.sync.dma_start(xt[:], xP8[j])
                    nc.scalar.dma_start(wt[:], wP8[j])
                    xts.append(xt)
                    wts.append(wt)
                for nch in range(NCH):
                    for half in range(2):
                        ps = ppool8.tile([64, D_OUT], f32, name="ps8")
                        col = nch * P + half * 64
                        for ji, xt in enumerate(xts):
                            lhsT = xt[:, :, col:col + 64]
                            last = ji == len(xts) - 1
                            for oc in range(3):
                                # 2KB-bank chains: bank0 = oc0+oc1
                                # (one start, one stop), bank1 = oc2.
                                nc.tensor.matmul(
                                    ps[:, oc * 256:(oc + 1) * 256],
                                    lhsT,
                                    wts[ji][:, :, oc * 256:(oc + 1) * 256],
                                    start=(ji == 0 and oc in (0, 2)),
                                    stop=(last and oc in (1, 2)),
                                    perf_mode=mybir.MatmulPerfMode.DoubleRow,
                                )
                        dst = cts[nch][half * 64:(half + 1) * 64, :]
                        if first:
                            nc.vector.tensor_scalar_mul(dst, ps[:], 1.0 / 128.0)
                        else:
                            nc.vector.scalar_tensor_tensor(
                                dst, ps[:], 1.0 / 128.0, dst,
                                op0=mybir.AluOpType.mult,
                                op1=mybir.AluOpType.add,
                            )
                first = False

            # fp16 supers (chunks of 128 d-rows)
            for s0 in range(0, c16, SUP16):
                js = list(range(s0, min(s0 + SUP16, c16)))
                xts, wts = [], []
                for j in js:
                    xt = x16pool.tile([P, N_SHARD], fp16, name="xt16")
                    wt = w16pool.tile([P, D_OUT], fp16, name="wt16")
                    nc.sync.dma_start(xt[:], xT16[j * P:(j + 1) * P, :])
                    nc.scalar.dma_start(wt[:], wT16[j * P:(j + 1) * P, :])
                    xts.append(xt)
                    wts.append(wt)
                for nch in range(NCH):
                    ps = ppool16.tile([P, D_OUT], f32, name="ps16")
                    for ji, xt in enumerate(xts):
                        lhsT = xt[:, nch * P:(nch + 1) * P]
                        st = ji == 0
                        sp = ji == len(xts) - 1
                        nc.tensor.matmul(ps[:, 0:512], lhsT,
                                         wts[ji][:, 0:512], start=st, stop=sp)
                        nc.tensor.matmul(ps[:, 512:D_OUT], lhsT,
                                         wts[ji][:, 512:D_OUT],
                                         start=st, stop=sp)
                    if first:
                        nc.vector.tensor_copy(cts[nch][:], ps[:])
                    else:
                        nc.vector.tensor_add(cts[nch][:], cts[nch][:], ps[:])
                first = False

            for nch in range(NCH):
                nc.sync.dma_start(out[nch * P:(nch + 1) * P, :], cts[nch][:])

    nc.compile()
    return nc


def _prep_inputs(x, W, c8):
    """Quantize + transpose + pack on the host. Returns per-core in_maps."""
    import ml_dtypes

    e4m3 = ml_dtypes.float8_e4m3
    d8 = c8 * 256
    in_maps = []
    if c8:
        w8 = np.ascontiguousarray((W[:, :d8].T * np.float32(128.0))
                                  .astype(e4m3)).reshape(c8, P, 2, D_OUT)
    if d8 < D_IN:
        w16 = np.ascontiguousarray(W[:, d8:].T.astype(np.float16))
    for c in range(N_CORES):
        xs = x[c * N_SHARD:(c + 1) * N_SHARD]
        m = {}
        if c8:
            m["xP8"] = np.ascontiguousarray(
                xs[:, :d8].T.astype(e4m3)).reshape(c8, P, 2, N_SHARD)
            m["wP8"] = w8
        if d8 < D_IN:
            m["xT16"] = np.ascontiguousarray(xs[:, d8:].T.astype(np.float16))
            m["wT16"] = w16
        in_maps.append(m)
    return in_maps


def _run_device(x, W, c8):
    global LAST_RESULTS
    from concourse.bass_utils import run_bass_kernel_spmd

    nc = _build_bass(c8)
    in_maps = _prep_inputs(x, W, c8)
    last_err = None
    for attempt in range(3):
        try:
            LAST_RESULTS = run_bass_kernel_spmd(
                nc, in_maps, core_ids=list(range(N_CORES)),
                tmpdir=os.environ.get("KERNEL_TRACE_DIR") or None,
            )
            break
        except Exception as e:  # transient device faults recover on retry
            last_err = e
            import time

            time.sleep(10)
    else:
        raise last_err
    return np.concatenate(
        [LAST_RESULTS.results[c]["out"] for c in range(N_CORES)], axis=0
    )


def kernel(x: np.ndarray, W: np.ndarray, b_pre: np.ndarray) -> np.ndarray:
    x = np.asarray(x, dtype=np.float32)
    W = np.asarray(W, dtype=np.float32)
    b_pre = np.asarray(b_pre, dtype=np.float32)

    # Fold the pre-bias on the host (exact no-op for b_pre == 0).
    if b_pre.any():
        x = x - b_pre[None, :]

    out = _run_device(x, W, C8)

    # Sampled sanity check (64 rows vs numpy fp64). The hybrid's
    # expected quantization error is ~1.6e-2 scale-relative; anything
    # above 2.6e-2 means the fp8 path misbehaved on this machine ->
    # redo in pure fp16 (expected ~3e-4).
    idx = np.arange(0, N_TOK, N_TOK // 64)
    ref = x[idx].astype(np.float64) @ W.astype(np.float64).T
    err = np.abs(out[idx] - ref).max() / (np.abs(ref).max() + 1e-30)
    if not np.isfinite(err) or err > 2.6e-2:
        out = _run_device(x, W, 0)
    return out


# revision 5
# speedup vs baseline: 1.2905x; 1.2905x over previous
"""Trainium2 Bass kernel for nn_Decoder: out = (x - b_pre) @ W^T.

Hybrid fp8-DoubleRow / fp16 kernel. See git history of the session for
the full design rationale. KERNEL_FP8_CHUNKS=0 gives pure fp16.
"""

import os
import sys

if "/opt/trn_rl_repo" not in sys.path:
    sys.path.insert(0, "/opt/trn_rl_repo")

import numpy as np

N_TOK = 8192
D_IN = 32768
D_OUT = 768
N_CORES = 8
N_SHARD = N_TOK // N_CORES          # 1024 token rows per core
P = 128

C8 = int(os.environ.get("KERNEL_FP8_CHUNKS", "24"))
D8 = C8 * 256
D16 = D_IN - D8
C16 = D16 // P                      # fp16 d-chunks of 128 rows

X8_BUFS = int(os.environ.get("KERNEL_X8_BUFS", "10"))
W8_BUFS = int(os.environ.get("KERNEL_W8_BUFS", "10"))
X16_BUFS = int(os.environ.get("KERNEL_X16_BUFS", "24"))
W16_BUFS = int(os.environ.get("KERNEL_W16_BUFS", "24"))
PS8_BUFS = int(os.environ.get("KERNEL_PS8_BUFS", "2"))
PS16_BUFS = int(os.environ.get("KERNEL_PS16_BUFS", "2"))
SUP8 = 8                            # fp8 chunks per super
SUP16 = int(os.environ.get("KERNEL_SUP16", "16"))  # fp16 chunks per super

LAST_RESULTS = None  # BassKernelResults of the most recent kernel() call


def _build_bass(c8):
    import concourse.mybir as mybir
    import concourse.tile as tile
    from concourse import bacc

    fp8 = mybir.dt.float8e4
    fp16 = mybir.dt.float16
    f32 = mybir.dt.float32
    c16 = (D_IN - c8 * 256) // P
    NCH = N_SHARD // P              # 8 output row-chunks of 128 tokens

    nc = bacc.Bacc(None, target_bir_lowering=False)
    if c8:
        xP8 = nc.dram_tensor("xP8", [c8, P, 2, N_SHARD], fp8,
                             kind="ExternalInput")
        wP8 = nc.dram_tensor("wP8", [c8, P, 2, D_OUT], fp8,
                             kind="ExternalInput")
    if c16:
        xT16 = nc.dram_tensor("xT16", [c16 * P, N_SHARD], fp16,
                              kind="ExternalInput")
        wT16 = nc.dram_tensor("wT16", [c16 * P, D_OUT], fp16,
                              kind="ExternalInput")
    out = nc.dram_tensor("out", [N_SHARD, D_OUT], f32,
                         kind="ExternalOutput")

    from contextlib import ExitStack

    with tile.TileContext(nc) as tc, ExitStack() as es:
        if c8:
            x8pool = es.enter_context(tc.tile_pool(name="x8", bufs=X8_BUFS))
            w8pool = es.enter_context(tc.tile_pool(name="w8", bufs=W8_BUFS))
            ppool8 = es.enter_context(
                tc.tile_pool(name="psum8", bufs=PS8_BUFS, space="PSUM"))
        if c16:
            x16pool = es.enter_context(
                tc.tile_pool(name="x16", bufs=X16_BUFS))
            w16pool = es.enter_context(
                tc.tile_pool(name="w16", bufs=W16_BUFS))
            ppool16 = es.enter_context(
                tc.tile_pool(name="psum16", bufs=PS16_BUFS, space="PSUM"))
        cpool = es.enter_context(tc.tile_pool(name="c", bufs=1))
        if True:
            cts = [
                cpool.tile([P, D_OUT], f32, name=f"c{i}") for i in range(NCH)
            ]
            first = True

            # fp8 DoubleRow supers (chunks of 256 d-rows)
            for s0 in range(0, c8, SUP8):
                js = list(range(s0, min(s0 + SUP8, c8)))
                xts, wts = [], []
                for j in js:
                    xt = x8pool.tile([P, 2, N_SHARD], fp8, name="xt8")
                    wt = w8pool.tile([P, 2, D_OUT], fp8, name="wt8")
                    nc.sync.dma_start(xt[:], xP8[j])
                    nc.scalar.dma_start(wt[:], wP8[j])
                    xts.append(xt)
                    wts.append(wt)
                for nch in range(NCH):
                    for half in range(2):
                        ps = ppool8.tile([64, D_OUT], f32, name="ps8")
                        col = nch * P + half * 64
                        for ji, xt in enumerate(xts):
                            lhsT = xt[:, :, col:col + 64]
                            last = ji == len(xts) - 1
                            for oc in range(3):
                                # 2KB-bank chains: bank0 = oc0+oc1
                                # (one start, one stop), bank1 = oc2.
                                nc.tensor.matmul(
                                    ps[:, oc * 256:(oc + 1) * 256],
                                    lhsT,
                                    wts[ji][:, :, oc * 256:(oc + 1) * 256],
                                    start=(ji == 0 and oc in (0, 2)),
                                    stop=(last and oc in (1, 2)),
                                    perf_mode=mybir.MatmulPerfMode.DoubleRow,
                                )
                        dst = cts[nch][half * 64:(half + 1) * 64, :]
                        if first:
                            nc.vector.tensor_scalar_mul(dst, ps[:], 1.0 / 128.0)
                        else:
                            nc.vector.scalar_tensor_tensor(
                                dst, ps[:], 1.0 / 128.0, dst,
                                op0=mybir.AluOpType.mult,
                                op1=mybir.AluOpType.add,
                            )
                first = False

            # fp16 supers (chunks of 128 d-rows)
            for s0 in range(0, c16, SUP16):
                js = list(range(s0, min(s0 + SUP16, c16)))
                xts, wts = [], []
                for j in js:
                    xt = x16pool.tile([P, N_SHARD], fp16, name="xt16")
                    wt = w16pool.tile([P, D_OUT], fp16, name="wt16")
                    nc.sync.dma_start(xt[:], xT16[j * P:(j + 1) * P, :])
                    nc.scalar.dma_start(wt[:], wT16[j * P:(j + 1) * P, :])
                    xts.append(xt)
                    wts.append(wt)
                for nch in range(NCH):
                    ps = ppool16.tile([P, D_OUT], f32, name="ps16")
                    for ji, xt in enumerate(xts):
                        lhsT = xt[:, nch * P:(nch + 1) * P]
                        st = ji == 0
                        sp = ji == len(xts) - 1
                        nc.tensor.matmul(ps[:, 0:512], lhsT,
                                         wts[ji][:, 0:512], start=st, stop=sp)
                        nc.tensor.matmul(ps[:, 512:D_OUT], lhsT,
                                         wts[ji][:, 512:D_OUT],
                                         start=st, stop=sp)
                    if first:
                        nc.vector.tensor_copy(cts[nch][:], ps[:])
                    else:
                        nc.vector.tensor_add(cts[nch][:], cts[nch][:], ps[:])
                first = False

            for nch in range(NCH):
                nc.sync.dma_start(out[nch * P:(nch + 1) * P, :], cts[nch][:])

    nc.compile()
    return nc


def _prep_inputs(x, W, c8):
    """Quantize + transpose + pack on the host. Returns per-core in_maps."""
    import ml_dtypes

    e4m3 = ml_dtypes.float8_e4m3
    d8 = c8 * 256
    in_maps = []
    if c8:
        w8 = np.ascontiguousarray((W[:, :d8].T * np.float32(128.0))
                                  .astype(e4m3)).reshape(c8, P, 2, D_OUT)
    if d8 < D_IN:
        w16 = np.ascontiguousarray(W[:, d8:].T.astype(np.float16))
    for c in range(N_CORES):
        xs = x[c * N_SHARD:(c + 1) * N_SHARD]
        m = {}
        if c8:
            m["xP8"] = np.ascontiguousarray(
                xs[:, :d8].T.astype(e4m3)).reshape(c8, P, 2, N_SHARD)
            m["wP8"] = w8
        if d8 < D_IN:
            m["xT16"] = np.ascontiguousarray(xs[:, d8:].T.astype(np.float16))
            m["wT16"] = w16
        in_maps.append(m)
    return in_maps


def _run_device(x, W, c8):
    global LAST_RESULTS
    from concourse.bass_utils import run_bass_kernel_spmd

    nc = _build_bass(c8)
    in_maps = _prep_inputs(x, W, c8)
    last_err = None
    for attempt in range(3):
        try:
            LAST_RESULTS = run_bass_kernel_spmd(
                nc, in_maps, core_ids=list(range(N_CORES)),
                tmpdir=os.environ.get("KERNEL_TRACE_DIR") or None,
            )
            break
        except Exception as e:  # transient device faults recover on retry
            last_err = e
            import time

            time.sleep(10)
    else:
        raise last_err
    return np.concatenate(
        [LAST_RESULTS.results[c]["out"] for c in range(N_CORES)], axis=0
    )


def kernel(x: np.ndarray, W: np.ndarray, b_pre: np.ndarray) -> np.ndarray:
    x = np.asarray(x, dtype=np.float32)
    W = np.asarray(W, dtype=np.float32)
    b_pre = np.asarray(b_pre, dtype=np.float32)

    # Fold the pre-bias on the host (exact no-op for b_pre == 0).
    if b_pre.any():
        x = x - b_pre[None, :]

    out = _run_device(x, W, C8)

    # Sampled sanity check (64 rows vs numpy fp64).
    idx = np.arange(0, N_TOK, N_TOK // 64)
    ref = x[idx].astype(np.float64) @ W.astype(np.float64).T
    err = np.abs(out[idx] - ref).max() / (np.abs(ref).max() + 1e-30)
    if not np.isfinite(err) or err > 2.6e-2:
        out = _run_device(x, W, 0)
    return out
